# revision 13
# baseline (speedup 1.0000x reference)
"""BiGRU (S=512, B=64, I=256, H=512, L=2) Trainium2 Bass kernel.

Strategy: 4-way batch split x 2-way direction split across 8 NeuronCores.
Cores 0-3 run the forward GRU chain (layers 0 and 1) for batch quarters
0-3; cores 4-7 run the backward chain (fed time-reversed input, so the
device program is identical on every core).

All input projections (gx = Wih @ x + b) are interleaved into the scans as
filler matmuls paced ~1.5 m-chunks per step, writing SBUF ring buffers the
scan consumes directly (no DRAM round trip).  The layer-0 hidden states are
exported per 8-step block (in the partner's processing order) and exchanged
with the direction partner via 8 chunked pairwise AllGathers issued during
the layer-0 scan, so the exchange overlaps compute.  Layer 1's projection
reads the gathered buffer (slot0 = forward half, slot1 = backward half —
the reference's concat order, identical on every core).

Scan step (the critical cycle): PE burst = 3 gx/bias injects + 48 Whh
matmuls in gate order r, n, z; Act stream sig_r, tanh, sig_z; DVE stream
tn, tn2, d, zd, hnew with h' = n + z*(h - n).
"""

import os
import sys
import numpy as np

for _p in ("/opt/trn_rl_repo", "/root/.axon_site/_ro/trn_rl_repo"):
    if os.path.isdir(_p) and _p not in sys.path:
        sys.path.insert(0, _p)

import ml_dtypes
from contextlib import ExitStack

import concourse.bass as bass
import concourse.tile as tile
from concourse import bacc, mybir
from concourse.bass import ts
from concourse.bass_utils import run_bass_kernel_spmd

BF16 = mybir.dt.bfloat16
F32 = mybir.dt.float32
AF = mybir.ActivationFunctionType
ALU = mybir.AluOpType

S, B, I, H, L = 512, 64, 256, 512, 2
G = 3 * H            # 1536 gate rows (r, z, n)
NCORE = 8
BQ = B // 4          # 16 batch per core
SB = S * BQ          # 8192 total moving columns
F = H // 128         # 4 h-fold chunks
M12 = G // 128       # 12 gate chunks
KI0 = I // 128       # 2 contraction chunks, layer-0 input proj
KI1 = 2 * H // 128   # 8 contraction chunks, layer-1 input proj
TBLK = 8             # scan block (steps); gx blocks are TBLK*BQ=128 cols
NBLK = S // TBLK     # 64 blocks per scan
NCHUNK = 8           # pairwise-AllGather chunks (8 blocks each)
CCOL = SB // NCHUNK  # 1024 columns per exchange chunk


def _p_mchunk(nc, ppsum, wsb, ki, rhs_ap_fn, gbias, m, out_ap):
    """One interleaved projection m-chunk: ki matmul pairs + bias + downcast."""
    ps = ppsum.tile([128, TBLK * BQ], F32)
    for k in range(ki):
        nc.tensor.matmul(ps[:], lhsT=wsb[:, k, ts(m, 128)], rhs=rhs_ap_fn(k),
                         start=(k == 0), stop=(k == ki - 1))
    if m % 2 == 0:
        nc.scalar.activation(out_ap, ps[:], AF.Identity, bias=gbias[:, m : m + 1])
    else:
        nc.vector.tensor_scalar_add(out_ap, ps[:], gbias[:, m : m + 1])


class _ProjFiller:
    """Paces interleaved projection m-chunks (and per-block ring DMAs) into
    the scan's PE stream.  Keeps emission exactly `lead` blocks ahead of the
    scan so the gx ring pool's WAR tracking stays valid."""

    def __init__(self, nc, tc, ctx, tag, ki, wsb, gbias, gxpool, block_rhs_fn,
                 lead=2):
        self.nc = nc
        self.ki = ki
        self.wsb = wsb
        self.gbias = gbias
        self.gxpool = gxpool
        self.block_rhs_fn = block_rhs_fn  # b -> (k -> AP [128, 128])
        self.ppsum = ctx.enter_context(
            tc.tile_pool(name=f"pps_{tag}", bufs=2, space="PSUM"))
        self.ring = {}
        self.next_b = 0
        self.next_m = 0
        self.credit = 0.0
        self.lead = lead
        self.rhs = None

    def _emit_one(self):
        b, m = self.next_b, self.next_m
        if b >= NBLK:
            return False
        if m == 0:
            self.ring[b] = self.gxpool.tile([128, M12, TBLK * BQ], BF16,
                                            name="gxring", tag="gxring")
            self.rhs = self.block_rhs_fn(b)
        _p_mchunk(self.nc, self.ppsum, self.wsb, self.ki, self.rhs,
                  self.gbias, m, self.ring[b][:, m, :])
        self.next_m += 1
        if self.next_m == M12:
            self.next_m = 0
            self.next_b += 1
        return True

    def prologue(self, nblocks):
        for _ in range(nblocks * M12):
            self._emit_one()

    def step(self, u, per_step=1.5):
        self.credit += per_step
        scan_block = u // TBLK
        while self.credit >= 1.0 and self.next_b <= scan_block + self.lead:
            if not self._emit_one():
                break
            self.credit -= 1.0

    def gx(self, u):
        return self.ring[u // TBLK][:, :, ts(u % TBLK, BQ)]


def _s_phase(ctx, tc, nc, whhT_dram, nbias_dram, filler, layer, y0own,
             y1T_dram, ident_dram, y0ex_dram, cc_fn):
    """512-step GRU scan with interleaved projection filler.

    layer 0: h -> y0own SBUF sequence + per-block y0ex DRAM export (in the
    partner's processing order) + chunked AllGather issue via cc_fn.
    layer 1: h -> y1sb staging -> y1T DRAM per block."""
    nc_ = nc
    tag = f"s{layer}"
    wpool = ctx.enter_context(tc.tile_pool(name=f"whh_{tag}", bufs=1))
    cpool = ctx.enter_context(tc.tile_pool(name=f"c_{tag}", bufs=1))
    psum = ctx.enter_context(tc.tile_pool(name=f"ps_{tag}", bufs=2, space="PSUM"))
    gp = ctx.enter_context(tc.tile_pool(name=f"g_{tag}", bufs=3))
    yp = ctx.enter_context(tc.tile_pool(name=f"y_{tag}", bufs=3))

    whh = wpool.tile([128, F, G], BF16)
    nc_.sync.dma_start(whh[:], whhT_dram.ap().rearrange("(k p) g -> p k g", p=128))
    ident = cpool.tile([128, 128], BF16)
    nc_.sync.dma_start(ident[:], ident_dram.ap())
    # nbias comes pre-broadcast from the host as bf16 [128, F*BQ]
    nbx = cpool.tile([128, F, BQ], BF16)
    nc_.sync.dma_start(nbx[:], nbias_dram.ap().rearrange("p (f b) -> p f b", b=BQ))
    zero_bf = cpool.tile([128, F, BQ], BF16)
    nc_.vector.memset(zero_bf[:], 0.0)

    y1_r = None
    if y1T_dram is not None:
        y1_r = y1T_dram.ap().rearrange("(f p) c -> p f c", p=128)

    h_prev = zero_bf[:]
    y1sb = None
    for u in range(S):
        j = u % TBLK
        if j == 0 and layer == 1:
            y1sb = yp.tile([128, F, TBLK * BQ], BF16, tag="y1sb")
        gx_t = filler.gx(u)

        ghr = psum.tile([128, F, BQ], F32, tag="ghr")
        ghn = psum.tile([128, F, BQ], F32, tag="ghn")
        ghz = psum.tile([128, F, BQ], F32, tag="ghz")
        # Injects first: no h dependency, so the in-order PE runs them during
        # the previous step's elementwise tail.  Then gate groups r, n, z.
        for gate, ps in (("r", ghr), ("n", ghn), ("z", ghz)):
            m0 = {"r": 0, "z": F, "n": 2 * F}[gate]
            inj = nbx[:] if gate == "n" else gx_t[:, m0 : m0 + F, :]
            nc_.tensor.matmul(ps[:], lhsT=ident[:], rhs=inj,
                              start=True, stop=False, skip_group_check=True)
        for gate, ps in (("r", ghr), ("n", ghn), ("z", ghz)):
            m0 = {"r": 0, "z": F, "n": 2 * F}[gate]
            for f in range(F):
                m = m0 + f
                for k in range(F):
                    nc_.tensor.matmul(ps[:, f, :], lhsT=whh[:, k, ts(m, 128)],
                                      rhs=h_prev[:, k, :],
                                      start=False, stop=(f == F - 1 and k == F - 1),
                                      skip_group_check=True)

        # Act stream: sig_r, tanh, sig_z.  DVE stream: tn, tn2, d, zd, hnew.
        # bf16 intermediates halve the per-op data time on the critical chain.
        r = gp.tile([128, F, BQ], BF16, tag="r")
        nc_.scalar.activation(r[:], ghr[:], AF.Sigmoid)
        # tn = r*ghn overwrites the ghr bank in place (sig_r has consumed
        # it); the PE then accumulates gxn onto it via an identity inject
        # (start=False), replacing the tn2 DVE op and its slow DVE->Act
        # crossing.  tanh reads the summed PSUM region.
        nc_.vector.tensor_tensor(ghr[:], r[:], ghn[:], ALU.mult)
        nc_.tensor.matmul(ghr[:], lhsT=ident[:], rhs=gx_t[:, 2 * F : 3 * F, :],
                          start=False, stop=True, skip_group_check=True)
        n = gp.tile([128, F, BQ], BF16, tag="n")
        nc_.scalar.activation(n[:], ghr[:], AF.Tanh)
        z = gp.tile([128, F, BQ], BF16, tag="z")
        nc_.scalar.activation(z[:], ghz[:], AF.Sigmoid)
        d = gp.tile([128, F, BQ], BF16, tag="d")
        nc_.vector.tensor_tensor(d[:], h_prev, n[:], ALU.subtract)
        zd = gp.tile([128, F, BQ], BF16, tag="zd")
        nc_.vector.tensor_tensor(zd[:], z[:], d[:], ALU.mult)

        if layer == 0:
            hslot = y0own[:, :, ts(u, BQ)]
        else:
            hslot = y1sb[:, :, ts(j, BQ)]
        nc_.vector.tensor_tensor(hslot, n[:], zd[:], ALU.add)
        h_prev = hslot

        # interleaved projection filler (runs in the PE tail window)
        filler.step(u)

        if j == TBLK - 1:
            blk = u // TBLK
            if layer == 0:
                # mirror this block of h states to y0ex, time-reversed at
                # BQ-block granularity (partner processing order), into the
                # chunk-major exchange layout [NCHUNK, H, CCOL]
                c = (NBLK - 1 - blk) // NCHUNK
                jj = (NBLK - 1 - blk) % NCHUNK
                y0e = y0ex_dram.ap()
                for f in range(F):
                    dst = bass.AP(
                        tensor=y0e.tensor,
                        offset=c * H * CCOL + f * 128 * CCOL
                        + jj * TBLK * BQ + (TBLK - 1) * BQ,
                        ap=[[CCOL, 128], [-BQ, TBLK], [1, BQ]],
                    )
                    src = y0own[:, f, ts(blk, TBLK * BQ)].rearrange(
                        "p (t b) -> p t b", b=BQ)
                    nc_.sync.dma_start(dst, src)
                if (blk + 1) % NCHUNK == 0:
                    cc_fn(NCHUNK - 1 - blk // NCHUNK)
            else:
                nc_.sync.dma_start(y1_r[:, :, ts(blk, TBLK * BQ)], y1sb[:])


def build_program(debug=False):
    nc = bacc.Bacc("TRN2", target_bir_lowering=False, debug=debug,
                   num_devices=NCORE)

    def din(name, shape, dt):
        return nc.dram_tensor(name, list(shape), dt, kind="ExternalInput")

    xT = din("xT", (I, SB), BF16)
    wih0T = din("wih0T", (I, G), BF16)
    whh0T = din("whh0T", (H, G), BF16)
    wih1T = din("wih1T", (2 * H, G), BF16)
    whh1T = din("whh1T", (H, G), BF16)
    gbias0 = din("gbias0", (128, M12), F32)
    gbias1 = din("gbias1", (128, M12), F32)
    nbias0 = din("nbias0", (128, F * BQ), BF16)
    nbias1 = din("nbias1", (128, F * BQ), BF16)
    ident = din("ident", (128, 128), BF16)

    y1T = nc.dram_tensor("y1T", [H, SB], BF16, kind="ExternalOutput")
    y0ex = nc.dram_tensor("y0ex", [NCHUNK, H, CCOL], BF16)
    y0g = nc.dram_tensor("y0g", [NCHUNK, 2, H, CCOL], BF16)
    y0loc = nc.dram_tensor("y0loc", [NCHUNK, H, CCOL], BF16)

    groups = [[2 * q, 2 * q + 1] for q in range(4)]

    with tile.TileContext(nc) as tc:
        with ExitStack() as ctx:
            # ---- persistent SBUF: input + projection weights ----
            xpool = ctx.enter_context(tc.tile_pool(name="xsb", bufs=1))
            xsb = xpool.tile([128, KI0, SB], BF16)
            nc.sync.dma_start(xsb[:], xT.ap().rearrange("(k p) c -> p k c", p=128))
            wp = ctx.enter_context(tc.tile_pool(name="wih", bufs=1))
            wih0sb = wp.tile([128, KI0, G], BF16)
            nc.sync.dma_start(wih0sb[:],
                              wih0T.ap().rearrange("(k p) g -> p k g", p=128))
            wih1sb = wp.tile([128, KI1, G], BF16)
            nc.sync.dma_start(wih1sb[:],
                              wih1T.ap().rearrange("(k p) g -> p k g", p=128))
            gbp = ctx.enter_context(tc.tile_pool(name="gb", bufs=1))
            gb0 = gbp.tile([128, M12], F32)
            nc.sync.dma_start(gb0[:], gbias0.ap())
            gb1 = gbp.tile([128, M12], F32)
            nc.sync.dma_start(gb1[:], gbias1.ap())

            y0pool = ctx.enter_context(tc.tile_pool(name="y0own", bufs=1))
            y0own = y0pool.tile([128, F, SB], BF16)

            rank = nc.gpsimd.cc_rank(groups)

            def cc_chunk(c):
                nc.gpsimd.collective_compute(
                    "AllGather", ALU.bypass,
                    ins=[y0ex.ap()[c]], outs=[y0g.ap()[c]],
                    replica_groups=groups,
                )
                # pull the partner's slot into the rank-free y0loc buffer
                with tc.If(rank < 1) as cmp:
                    nc.gpsimd.dma_start(y0loc.ap()[c], y0g.ap()[c, 1])
                with cmp.Else():
                    nc.gpsimd.dma_start(y0loc.ap()[c], y0g.ap()[c, 0])

            # ---- S0 with interleaved P0 ----
            with ExitStack() as sctx:
                gx0pool = sctx.enter_context(tc.tile_pool(name="gx0", bufs=4))

                def rhs0(b):
                    return lambda k: xsb[:, k, ts(b, TBLK * BQ)]

                f0 = _ProjFiller(nc, tc, sctx, "p0", KI0, wih0sb, gb0,
                                 gx0pool, rhs0)
                f0.prologue(2)
                _s_phase(sctx, tc, nc, whh0T, nbias0, f0, 0, y0own, None,
                         ident, y0ex, cc_chunk)

            # ---- S1 with interleaved P1 (reads gathered y0g) ----
            with ExitStack() as sctx:
                gx1pool = sctx.enter_context(tc.tile_pool(name="gx1", bufs=4))
                prpool = sctx.enter_context(tc.tile_pool(name="pring", bufs=3))
                pring = {}

                def rhs1(b):
                    t = prpool.tile([128, F, TBLK * BQ], BF16, name="pring",
                                     tag="pring")
                    pring[b] = t
                    c, jj = b // NCHUNK, b % NCHUNK
                    src = bass.AP(
                        tensor=y0loc.ap().tensor,
                        offset=c * H * CCOL + jj * TBLK * BQ,
                        ap=[[CCOL, 128], [128 * CCOL, F], [1, TBLK * BQ]],
                    )
                    nc.sync.dma_start(t[:], src)
                    return lambda k: (y0own[:, k, ts(b, TBLK * BQ)] if k < F
                                      else t[:, k - F, :])

                f1 = _ProjFiller(nc, tc, sctx, "p1", KI1, wih1sb, gb1,
                                 gx1pool, rhs1)
                f1.prologue(2)
                _s_phase(sctx, tc, nc, whh1T, nbias1, f1, 1, None, y1T,
                         ident, None, None)

    nc.compile()
    return nc


_PROGRAM_CACHE = {}


def _get_program():
    if "nc" not in _PROGRAM_CACHE:
        _PROGRAM_CACHE["nc"] = build_program()
    return _PROGRAM_CACHE["nc"]


def _host_inputs(inputs):
    """Build the 8 per-core input maps from the full problem inputs."""
    bf = ml_dtypes.bfloat16
    x = np.asarray(inputs["input"], np.float32)            # (S, B, I)
    in_maps = []
    for c in range(NCORE):
        fwd = c % 2 == 0
        q = c // 2
        d = "f" if fwd else "b"
        xq = x[:, q * BQ:(q + 1) * BQ, :]
        if not fwd:
            xq = xq[::-1]
        xTv = np.ascontiguousarray(xq.transpose(2, 0, 1).reshape(I, SB))

        def wT(wname):
            return np.ascontiguousarray(np.asarray(inputs[wname], np.float32).T)

        wih0 = wT(f"Wih_{d}0")        # (I, G)
        whh0 = wT(f"Whh_{d}0")        # (H, G)
        wih1_full = wT(f"Wih_{d}1")   # (2H, G); rows = y0 features [hf | hb]
        own_sl = slice(0, H) if fwd else slice(H, 2 * H)
        par_sl = slice(H, 2 * H) if fwd else slice(0, H)
        wih1 = np.concatenate([wih1_full[own_sl], wih1_full[par_sl]], axis=0)
        whh1 = wT(f"Whh_{d}1")

        def gbias(layer):
            bih = np.asarray(inputs[f"bih_{d}{layer}"], np.float32)
            bhh = np.asarray(inputs[f"bhh_{d}{layer}"], np.float32)
            gb = np.concatenate([bih[:2 * H] + bhh[:2 * H], bih[2 * H:]])
            return np.ascontiguousarray(gb.reshape(M12, 128).T)  # [128, M12]

        def nbias(layer):
            bhh = np.asarray(inputs[f"bhh_{d}{layer}"], np.float32)
            nb = bhh[2 * H:].reshape(F, 128).T  # [128, F]
            return np.ascontiguousarray(
                np.broadcast_to(nb[:, :, None], (128, F, BQ)).reshape(
                    128, F * BQ)).astype(bf)

        in_maps.append({
            "xT": xTv.astype(bf),
            "wih0T": wih0.astype(bf), "whh0T": whh0.astype(bf),
            "wih1T": wih1.astype(bf), "whh1T": whh1.astype(bf),
            "gbias0": gbias(0), "gbias1": gbias(1),
            "nbias0": nbias(0), "nbias1": nbias(1),
            "ident": np.eye(128, dtype=bf),
        })
    return in_maps


def kernel(**inputs) -> np.ndarray:
    nc = _get_program()
    in_maps = _host_inputs(inputs)
    trace = bool(int(os.environ.get("BIGRU_TRACE", "0")))
    kw = {}
    if trace and os.environ.get("BIGRU_TRACE_DIR"):
        kw["tmpdir"] = os.environ["BIGRU_TRACE_DIR"]
    res = run_bass_kernel_spmd(nc, in_maps, list(range(NCORE)), trace=trace, **kw)
    if trace and res.exec_time_ns is not None:
        print(f"HW exec time: {res.exec_time_ns} ns")
        _PROGRAM_CACHE["exec_time_ns"] = res.exec_time_ns
        _PROGRAM_CACHE["profile_json"] = res.profile_json

    out = np.empty((S, B, 2 * H), np.float32)
    for c in range(NCORE):
        fwd = c % 2 == 0
        q = c // 2
        y = np.asarray(res.results[c]["y1T"], dtype=np.float32)
        y = y.reshape(H, S, BQ).transpose(1, 2, 0)  # (S, BQ, H)
        if not fwd:
            y = y[::-1]
        out[:, q * BQ:(q + 1) * BQ, (0 if fwd else H):(H if fwd else 2 * H)] = y
    return out


# revision 14
# speedup vs baseline: 1.0854x; 1.0854x over previous
"""BiGRU (S=512, B=64, I=256, H=512, L=2) Trainium2 Bass kernel.

Strategy: 4-way batch split x 2-way direction split across 8 NeuronCores.
Cores 0-3 run the forward GRU chain (layers 0 and 1) for batch quarters
0-3; cores 4-7 run the backward chain (fed time-reversed input, so the
device program is identical on every core).

All input projections (gx = Wih @ x + b) are interleaved into the scans as
filler matmuls paced ~1.5 m-chunks per step, writing SBUF ring buffers the
scan consumes directly (no DRAM round trip).  The layer-0 hidden states are
exported per 8-step block (in the partner's processing order) and exchanged
with the direction partner via 8 chunked pairwise AllGathers issued during
the layer-0 scan, so the exchange overlaps compute.  Layer 1's projection
reads the gathered buffer (slot0 = forward half, slot1 = backward half —
the reference's concat order, identical on every core).

Scan step (the critical cycle): PE burst = 3 gx/bias injects + 48 Whh
matmuls in gate order r, n, z; Act stream sig_r, tanh, sig_z; DVE stream
tn, tn2, d, zd, hnew with h' = n + z*(h - n).
"""

import os
import sys
import numpy as np

for _p in ("/opt/trn_rl_repo", "/root/.axon_site/_ro/trn_rl_repo"):
    if os.path.isdir(_p) and _p not in sys.path:
        sys.path.insert(0, _p)

import ml_dtypes
from contextlib import ExitStack

import concourse.bass as bass
import concourse.tile as tile
from concourse import bacc, mybir
from concourse.bass import ts
from concourse.bass_utils import run_bass_kernel_spmd

BF16 = mybir.dt.bfloat16
F32 = mybir.dt.float32
AF = mybir.ActivationFunctionType
ALU = mybir.AluOpType

S, B, I, H, L = 512, 64, 256, 512, 2
G = 3 * H            # 1536 gate rows (r, z, n)
NCORE = 8
BQ = B // 4          # 16 batch per core
SB = S * BQ          # 8192 total moving columns
F = H // 128         # 4 h-fold chunks
M12 = G // 128       # 12 gate chunks
KI0 = I // 128       # 2 contraction chunks, layer-0 input proj
KI1 = 2 * H // 128   # 8 contraction chunks, layer-1 input proj
TBLK = 8             # scan block (steps); gx blocks are TBLK*BQ=128 cols
NBLK = S // TBLK     # 64 blocks per scan
NCHUNK = 8           # pairwise-AllGather chunks (8 blocks each)
CCOL = SB // NCHUNK  # 1024 columns per exchange chunk


def _p_mchunk(nc, ppsum, wsb, ki, rhs_ap_fn, gbias, m, out_ap):
    """One interleaved projection m-chunk: ki matmul pairs + bias + downcast."""
    ps = ppsum.tile([128, TBLK * BQ], F32)
    for k in range(ki):
        nc.tensor.matmul(ps[:], lhsT=wsb[:, k, ts(m, 128)], rhs=rhs_ap_fn(k),
                         start=(k == 0), stop=(k == ki - 1))
    if m % 2 == 0:
        nc.scalar.activation(out_ap, ps[:], AF.Identity, bias=gbias[:, m : m + 1])
    else:
        nc.vector.tensor_scalar_add(out_ap, ps[:], gbias[:, m : m + 1])


class _ProjFiller:
    """Paces interleaved projection m-chunks (and per-block ring DMAs) into
    the scan's PE stream.  Keeps emission exactly `lead` blocks ahead of the
    scan so the gx ring pool's WAR tracking stays valid."""

    def __init__(self, nc, tc, ctx, tag, ki, wsb, gbias, gxpool, block_rhs_fn,
                 lead=2):
        self.nc = nc
        self.ki = ki
        self.wsb = wsb
        self.gbias = gbias
        self.gxpool = gxpool
        self.block_rhs_fn = block_rhs_fn  # b -> (k -> AP [128, 128])
        self.ppsum = ctx.enter_context(
            tc.tile_pool(name=f"pps_{tag}", bufs=2, space="PSUM"))
        self.ring = {}
        self.next_b = 0
        self.next_m = 0
        self.credit = 0.0
        self.lead = lead
        self.rhs = None

    def _emit_one(self):
        b, m = self.next_b, self.next_m
        if b >= NBLK:
            return False
        if m == 0:
            self.ring[b] = self.gxpool.tile([128, M12, TBLK * BQ], BF16,
                                            name="gxring", tag="gxring")
            self.rhs = self.block_rhs_fn(b)
        _p_mchunk(self.nc, self.ppsum, self.wsb, self.ki, self.rhs,
                  self.gbias, m, self.ring[b][:, m, :])
        self.next_m += 1
        if self.next_m == M12:
            self.next_m = 0
            self.next_b += 1
        return True

    def prologue(self, nblocks):
        for _ in range(nblocks * M12):
            self._emit_one()

    def step(self, u, per_step=1.5):
        self.credit += per_step
        scan_block = u // TBLK
        while self.credit >= 1.0 and self.next_b <= scan_block + self.lead:
            if not self._emit_one():
                break
            self.credit -= 1.0

    def gx(self, u):
        return self.ring[u // TBLK][:, :, ts(u % TBLK, BQ)]


def _s_phase(ctx, tc, nc, whhT_dram, nbias_dram, filler, layer, y0own,
             y1T_dram, ident_dram, y0ex_dram, cc_fn):
    """512-step GRU scan with interleaved projection filler.

    layer 0: h -> y0own SBUF sequence + per-block y0ex DRAM export (in the
    partner's processing order) + chunked AllGather issue via cc_fn.
    layer 1: h -> y1sb staging -> y1T DRAM per block."""
    nc_ = nc
    tag = f"s{layer}"
    wpool = ctx.enter_context(tc.tile_pool(name=f"whh_{tag}", bufs=1))
    cpool = ctx.enter_context(tc.tile_pool(name=f"c_{tag}", bufs=1))
    psum = ctx.enter_context(tc.tile_pool(name=f"ps_{tag}", bufs=2, space="PSUM"))
    gp = ctx.enter_context(tc.tile_pool(name=f"g_{tag}", bufs=3))
    yp = ctx.enter_context(tc.tile_pool(name=f"y_{tag}", bufs=3))

    whh = wpool.tile([128, F, G], BF16)
    nc_.sync.dma_start(whh[:], whhT_dram.ap().rearrange("(k p) g -> p k g", p=128))
    ident = cpool.tile([128, 128], BF16)
    nc_.sync.dma_start(ident[:], ident_dram.ap())
    # nbias comes pre-broadcast from the host as bf16 [128, F*BQ]
    nbx = cpool.tile([128, F, BQ], BF16)
    nc_.sync.dma_start(nbx[:], nbias_dram.ap().rearrange("p (f b) -> p f b", b=BQ))
    zero_bf = cpool.tile([128, F, BQ], BF16)
    nc_.vector.memset(zero_bf[:], 0.0)

    y1_r = None
    if y1T_dram is not None:
        y1_r = y1T_dram.ap().rearrange("(f p) c -> p f c", p=128)

    h_prev = zero_bf[:]
    y1sb = None
    for u in range(S):
        j = u % TBLK
        if j == 0 and layer == 1:
            y1sb = yp.tile([128, F, TBLK * BQ], BF16, tag="y1sb")
        gx_t = filler.gx(u)

        ghr = psum.tile([128, F, BQ], F32, tag="ghr")
        ghn = psum.tile([128, F, BQ], F32, tag="ghn")
        ghz = psum.tile([128, F, BQ], F32, tag="ghz")
        # Injects first: no h dependency, so the in-order PE runs them during
        # the previous step's elementwise tail.  Then gate groups r, n, z.
        for gate, ps in (("r", ghr), ("n", ghn), ("z", ghz)):
            m0 = {"r": 0, "z": F, "n": 2 * F}[gate]
            inj = nbx[:] if gate == "n" else gx_t[:, m0 : m0 + F, :]
            nc_.tensor.matmul(ps[:], lhsT=ident[:], rhs=inj,
                              start=True, stop=False, skip_group_check=True)
        for gate, ps in (("r", ghr), ("n", ghn), ("z", ghz)):
            m0 = {"r": 0, "z": F, "n": 2 * F}[gate]
            for f in range(F):
                m = m0 + f
                for k in range(F):
                    nc_.tensor.matmul(ps[:, f, :], lhsT=whh[:, k, ts(m, 128)],
                                      rhs=h_prev[:, k, :],
                                      start=False, stop=(f == F - 1 and k == F - 1),
                                      skip_group_check=True)

        # Act stream: sig_r, tanh, sig_z.  DVE stream: tn, tn2, d, zd, hnew.
        # bf16 intermediates halve the per-op data time on the critical chain.
        r = gp.tile([128, F, BQ], BF16, tag="r")
        nc_.scalar.activation(r[:], ghr[:], AF.Sigmoid)
        tn = gp.tile([128, F, BQ], F32, tag="tn")
        nc_.vector.tensor_tensor(tn[:], r[:], ghn[:], ALU.mult)
        tn2 = gp.tile([128, F, BQ], BF16, tag="tn2")
        nc_.vector.tensor_tensor(tn2[:], tn[:], gx_t[:, 2 * F : 3 * F, :], ALU.add)
        n = gp.tile([128, F, BQ], BF16, tag="n")
        nc_.scalar.activation(n[:], tn2[:], AF.Tanh)
        z = gp.tile([128, F, BQ], BF16, tag="z")
        nc_.scalar.activation(z[:], ghz[:], AF.Sigmoid)
        d = gp.tile([128, F, BQ], BF16, tag="d")
        nc_.vector.tensor_tensor(d[:], h_prev, n[:], ALU.subtract)
        zd = gp.tile([128, F, BQ], BF16, tag="zd")
        nc_.vector.tensor_tensor(zd[:], z[:], d[:], ALU.mult)

        if layer == 0:
            hslot = y0own[:, :, ts(u, BQ)]
        else:
            hslot = y1sb[:, :, ts(j, BQ)]
        nc_.vector.tensor_tensor(hslot, n[:], zd[:], ALU.add)
        h_prev = hslot

        # interleaved projection filler (runs in the PE tail window)
        filler.step(u)

        if j == TBLK - 1:
            blk = u // TBLK
            if layer == 0:
                # mirror this block of h states to y0ex, time-reversed at
                # BQ-block granularity (partner processing order), into the
                # chunk-major exchange layout [NCHUNK, H, CCOL]
                c = (NBLK - 1 - blk) // NCHUNK
                jj = (NBLK - 1 - blk) % NCHUNK
                y0e = y0ex_dram.ap()
                for f in range(F):
                    dst = bass.AP(
                        tensor=y0e.tensor,
                        offset=c * H * CCOL + f * 128 * CCOL
                        + jj * TBLK * BQ + (TBLK - 1) * BQ,
                        ap=[[CCOL, 128], [-BQ, TBLK], [1, BQ]],
                    )
                    src = y0own[:, f, ts(blk, TBLK * BQ)].rearrange(
                        "p (t b) -> p t b", b=BQ)
                    nc_.sync.dma_start(dst, src)
                if (blk + 1) % NCHUNK == 0:
                    cc_fn(NCHUNK - 1 - blk // NCHUNK)
            else:
                nc_.sync.dma_start(y1_r[:, :, ts(blk, TBLK * BQ)], y1sb[:])


def build_program(debug=False):
    nc = bacc.Bacc("TRN2", target_bir_lowering=False, debug=debug,
                   num_devices=NCORE)

    def din(name, shape, dt):
        return nc.dram_tensor(name, list(shape), dt, kind="ExternalInput")

    xT = din("xT", (I, SB), BF16)
    wih0T = din("wih0T", (I, G), BF16)
    whh0T = din("whh0T", (H, G), BF16)
    wih1T = din("wih1T", (2 * H, G), BF16)
    whh1T = din("whh1T", (H, G), BF16)
    gbias0 = din("gbias0", (128, M12), F32)
    gbias1 = din("gbias1", (128, M12), F32)
    nbias0 = din("nbias0", (128, F * BQ), BF16)
    nbias1 = din("nbias1", (128, F * BQ), BF16)
    ident = din("ident", (128, 128), BF16)

    y1T = nc.dram_tensor("y1T", [H, SB], BF16, kind="ExternalOutput")
    y0ex = nc.dram_tensor("y0ex", [NCHUNK, H, CCOL], BF16)
    y0g = nc.dram_tensor("y0g", [NCHUNK, 2, H, CCOL], BF16)
    y0loc = nc.dram_tensor("y0loc", [NCHUNK, H, CCOL], BF16)

    groups = [[2 * q, 2 * q + 1] for q in range(4)]

    with tile.TileContext(nc) as tc:
        with ExitStack() as ctx:
            # ---- persistent SBUF: input + projection weights ----
            xpool = ctx.enter_context(tc.tile_pool(name="xsb", bufs=1))
            xsb = xpool.tile([128, KI0, SB], BF16)
            nc.sync.dma_start(xsb[:], xT.ap().rearrange("(k p) c -> p k c", p=128))
            wp = ctx.enter_context(tc.tile_pool(name="wih", bufs=1))
            wih0sb = wp.tile([128, KI0, G], BF16)
            nc.sync.dma_start(wih0sb[:],
                              wih0T.ap().rearrange("(k p) g -> p k g", p=128))
            wih1sb = wp.tile([128, KI1, G], BF16)
            nc.sync.dma_start(wih1sb[:],
                              wih1T.ap().rearrange("(k p) g -> p k g", p=128))
            gbp = ctx.enter_context(tc.tile_pool(name="gb", bufs=1))
            gb0 = gbp.tile([128, M12], F32)
            nc.sync.dma_start(gb0[:], gbias0.ap())
            gb1 = gbp.tile([128, M12], F32)
            nc.sync.dma_start(gb1[:], gbias1.ap())

            y0pool = ctx.enter_context(tc.tile_pool(name="y0own", bufs=1))
            y0own = y0pool.tile([128, F, SB], BF16)

            rank = nc.gpsimd.cc_rank(groups)

            def cc_chunk(c):
                nc.gpsimd.collective_compute(
                    "AllGather", ALU.bypass,
                    ins=[y0ex.ap()[c]], outs=[y0g.ap()[c]],
                    replica_groups=groups,
                )
                # pull the partner's slot into the rank-free y0loc buffer
                with tc.If(rank < 1) as cmp:
                    nc.gpsimd.dma_start(y0loc.ap()[c], y0g.ap()[c, 1])
                with cmp.Else():
                    nc.gpsimd.dma_start(y0loc.ap()[c], y0g.ap()[c, 0])

            # ---- S0 with interleaved P0 ----
            with ExitStack() as sctx:
                gx0pool = sctx.enter_context(tc.tile_pool(name="gx0", bufs=4))

                def rhs0(b):
                    return lambda k: xsb[:, k, ts(b, TBLK * BQ)]

                f0 = _ProjFiller(nc, tc, sctx, "p0", KI0, wih0sb, gb0,
                                 gx0pool, rhs0)
                f0.prologue(2)
                _s_phase(sctx, tc, nc, whh0T, nbias0, f0, 0, y0own, None,
                         ident, y0ex, cc_chunk)

            # ---- S1 with interleaved P1 (reads gathered y0g) ----
            with ExitStack() as sctx:
                gx1pool = sctx.enter_context(tc.tile_pool(name="gx1", bufs=4))
                prpool = sctx.enter_context(tc.tile_pool(name="pring", bufs=3))
                pring = {}

                def rhs1(b):
                    t = prpool.tile([128, F, TBLK * BQ], BF16, name="pring",
                                     tag="pring")
                    pring[b] = t
                    c, jj = b // NCHUNK, b % NCHUNK
                    src = bass.AP(
                        tensor=y0loc.ap().tensor,
                        offset=c * H * CCOL + jj * TBLK * BQ,
                        ap=[[CCOL, 128], [128 * CCOL, F], [1, TBLK * BQ]],
                    )
                    nc.sync.dma_start(t[:], src)
                    return lambda k: (y0own[:, k, ts(b, TBLK * BQ)] if k < F
                                      else t[:, k - F, :])

                f1 = _ProjFiller(nc, tc, sctx, "p1", KI1, wih1sb, gb1,
                                 gx1pool, rhs1)
                f1.prologue(2)
                _s_phase(sctx, tc, nc, whh1T, nbias1, f1, 1, None, y1T,
                         ident, None, None)

    nc.compile()
    return nc


_PROGRAM_CACHE = {}


def _get_program():
    if "nc" not in _PROGRAM_CACHE:
        _PROGRAM_CACHE["nc"] = build_program()
    return _PROGRAM_CACHE["nc"]


def _host_inputs(inputs):
    """Build the 8 per-core input maps from the full problem inputs."""
    bf = ml_dtypes.bfloat16
    x = np.asarray(inputs["input"], np.float32)            # (S, B, I)
    in_maps = []
    for c in range(NCORE):
        fwd = c % 2 == 0
        q = c // 2
        d = "f" if fwd else "b"
        xq = x[:, q * BQ:(q + 1) * BQ, :]
        if not fwd:
            xq = xq[::-1]
        xTv = np.ascontiguousarray(xq.transpose(2, 0, 1).reshape(I, SB))

        def wT(wname):
            return np.ascontiguousarray(np.asarray(inputs[wname], np.float32).T)

        wih0 = wT(f"Wih_{d}0")        # (I, G)
        whh0 = wT(f"Whh_{d}0")        # (H, G)
        wih1_full = wT(f"Wih_{d}1")   # (2H, G); rows = y0 features [hf | hb]
        own_sl = slice(0, H) if fwd else slice(H, 2 * H)
        par_sl = slice(H, 2 * H) if fwd else slice(0, H)
        wih1 = np.concatenate([wih1_full[own_sl], wih1_full[par_sl]], axis=0)
        whh1 = wT(f"Whh_{d}1")

        def gbias(layer):
            bih = np.asarray(inputs[f"bih_{d}{layer}"], np.float32)
            bhh = np.asarray(inputs[f"bhh_{d}{layer}"], np.float32)
            gb = np.concatenate([bih[:2 * H] + bhh[:2 * H], bih[2 * H:]])
            return np.ascontiguousarray(gb.reshape(M12, 128).T)  # [128, M12]

        def nbias(layer):
            bhh = np.asarray(inputs[f"bhh_{d}{layer}"], np.float32)
            nb = bhh[2 * H:].reshape(F, 128).T  # [128, F]
            return np.ascontiguousarray(
                np.broadcast_to(nb[:, :, None], (128, F, BQ)).reshape(
                    128, F * BQ)).astype(bf)

        in_maps.append({
            "xT": xTv.astype(bf),
            "wih0T": wih0.astype(bf), "whh0T": whh0.astype(bf),
            "wih1T": wih1.astype(bf), "whh1T": whh1.astype(bf),
            "gbias0": gbias(0), "gbias1": gbias(1),
            "nbias0": nbias(0), "nbias1": nbias(1),
            "ident": np.eye(128, dtype=bf),
        })
    return in_maps


def kernel(**inputs) -> np.ndarray:
    nc = _get_program()
    in_maps = _host_inputs(inputs)
    trace = bool(int(os.environ.get("BIGRU_TRACE", "0")))
    kw = {}
    if trace and os.environ.get("BIGRU_TRACE_DIR"):
        kw["tmpdir"] = os.environ["BIGRU_TRACE_DIR"]
    res = run_bass_kernel_spmd(nc, in_maps, list(range(NCORE)), trace=trace, **kw)
    if trace and res.exec_time_ns is not None:
        print(f"HW exec time: {res.exec_time_ns} ns")
        _PROGRAM_CACHE["exec_time_ns"] = res.exec_time_ns
        _PROGRAM_CACHE["profile_json"] = res.profile_json

    out = np.empty((S, B, 2 * H), np.float32)
    for c in range(NCORE):
        fwd = c % 2 == 0
        q = c // 2
        y = np.asarray(res.results[c]["y1T"], dtype=np.float32)
        y = y.reshape(H, S, BQ).transpose(1, 2, 0)  # (S, BQ, H)
        if not fwd:
            y = y[::-1]
        out[:, q * BQ:(q + 1) * BQ, (0 if fwd else H):(H if fwd else 2 * H)] = y
    return out
